# revision 13
# baseline (speedup 1.0000x reference)
"""AdaptiveMLP (moe_routing) Trainium2 kernel — 8 NeuronCores, data-parallel.

Reference computation:
    gate = softmax(MLP_gate(mean_s(x)))          # [B,3] per-batch weights
    out  = sum_p w[b,p] * MLP_p(x[b])            # 3 expert MLPs (SiLU)
with w = gate masked to the argmax expert when confident (maxw >= 0.6).

Strategy:
  - The gate network is tiny ([4,2048] @ [2048,512] @ [512,3]); compute it on
    the host exactly as the reference does, producing the [B,3] combine
    weights `w`. The device then always computes all three expert MLPs and
    combines them with `w` — algebraically identical to the reference for
    both the confident and non-confident paths.
  - Data-parallel over tokens: 8 cores x 1024 tokens (core c takes batch
    c//2, sequence half c%2). No collectives; each core streams all expert
    weights once (bf16) while keeping its activations resident in SBUF.
  - bf16 compute on the TensorEngine (1 cycle/row vs 4 for fp32), f32
    accumulation in PSUM and for the output.

Per-core dataflow (tokens T=1024, H=2048, expert intermediate I in
{8192,4096,8192} processed in chunks of 512):
    xT [H,T] resident in SBUF (transposed+bf16 on host).
    per chunk: psum_h[i128,t512] = sum_hi W1[hi,i128].T @ xT[hi,t512]
               hT = Silu(psum_h + b1)                  (ScalarE, bf16)
               psum_o[t128,h512] = sum_j hT[j,t128].T @ W2[j,h512]
               out_acc += w[expert] * psum_o           (VectorE, f32)
    out_acc initialised with sum_p w[b,p]*b2_p (host-combined bias).
"""

import os
import sys

import numpy as np

sys.path.insert(0, "/opt/trn_rl_repo")

import ml_dtypes

BF16 = ml_dtypes.bfloat16

B, S, H = 4, 2048, 2048
I_FULL, I_COMP = 8192, 4096
N_CORES = 8
TOK = (B * S) // N_CORES  # 1024 tokens per core
CONF_THRESH = 0.6
CHUNK_I = 512  # intermediate-dim chunk processed per inner iteration

_compiled = {}


def _enable_ldw_opt():
    """Walrus can elide LDWEIGHTS for consecutive matmuls that share the
    same stationary operand, but concourse hardcodes --enable-ldw-opt=false.
    Rewrite the flag on the walrus command line; correctness is verified
    against the reference each run."""
    from concourse import bass_utils as _bu

    if getattr(_bu, "_ldw_opt_patched", False):
        return
    orig = _bu.run_command

    # NOTE: --enable-ldw-opt=true crashes this walrus build's codegen
    # (visitInstLdweights, CoreV3GenImpl.cpp:694) — leave the flag alone.
    _bu._ldw_opt_patched = True
    del orig


def _build_graph():
    import concourse.tile as tile
    from concourse import bacc, mybir

    _enable_ldw_opt()

    nc = bacc.Bacc("TRN2", target_bir_lowering=False, debug=False,
                   num_devices=N_CORES)
    f32 = mybir.dt.float32
    bf16 = mybir.dt.bfloat16

    xT = nc.dram_tensor("xT", [H, TOK], bf16, kind="ExternalInput").ap()
    w1 = {}
    w2 = {}
    for name, i_dim in (("f", I_FULL), ("c", I_COMP), ("s", I_FULL)):
        w1[name] = nc.dram_tensor(f"w1{name}", [H, i_dim], bf16,
                                  kind="ExternalInput").ap()
        w2[name] = nc.dram_tensor(f"w2{name}", [i_dim, H], bf16,
                                  kind="ExternalInput").ap()
    b1 = {
        "f": nc.dram_tensor("b1f", [128, I_FULL // 128], f32,
                            kind="ExternalInput").ap(),
        "c": nc.dram_tensor("b1c", [128, I_COMP // 128], f32,
                            kind="ExternalInput").ap(),
        "s": nc.dram_tensor("b1s", [128, I_FULL // 128], f32,
                            kind="ExternalInput").ap(),
    }
    bias2 = nc.dram_tensor("bias2", [128, H], f32, kind="ExternalInput").ap()
    wvec = nc.dram_tensor("wvec", [128, 4], f32, kind="ExternalInput").ap()
    out = nc.dram_tensor("out", [TOK, H], f32, kind="ExternalOutput").ap()

    N_HI = H // 128           # 16 contraction tiles for GEMM1
    N_HI_POOL = N_HI
    N_TT = TOK // 128         # 8 token tiles
    N_HH = H // 512           # 4 output-H tiles
    N_TC = TOK // 512         # 2 token chunks for GEMM1 moving operand
    N_JB = CHUNK_I // 128     # 4 I-blocks per chunk

    with tile.TileContext(nc) as tc:
        with (
            tc.tile_pool(name="singles", bufs=1) as singles,
            tc.tile_pool(name="w1p", bufs=2 * N_HI_POOL) as w1p,
            tc.tile_pool(name="w2p", bufs=2 * 4) as w2p,
            tc.tile_pool(name="hp", bufs=2) as hp,
            tc.tile_pool(name="ps1", bufs=4, space="PSUM") as ps1p,
            tc.tile_pool(name="ps2", bufs=4, space="PSUM") as ps2p,
        ):
            # Per-hi slice tiles so the first matmuls only wait on the
            # slices they read (head-latency: overlap DMA with compute).
            # The DMAs for xT are interleaved with the first chunk's W1
            # slices below so the first GEMM1 group starts ~3us in.
            xT_r = xT.rearrange("(hi p) t -> p hi t", p=128)
            xT_sb = [singles.tile([128, TOK], bf16, tag=f"xT{hi}",
                                  name=f"xT{hi}") for hi in range(N_HI)]
            bias2_sb = singles.tile([128, H], f32)
            nc.sync.dma_start(out=bias2_sb[:], in_=bias2[:])
            wvec_sb = singles.tile([128, 4], f32)
            nc.sync.dma_start(out=wvec_sb[:], in_=wvec[:])
            b1_sb = {}
            for e in ("f", "c", "s"):
                t = singles.tile([128, b1[e].shape[1]], f32, tag=f"b1{e}")
                nc.sync.dma_start(out=t[:], in_=b1[e][:])
                b1_sb[e] = t
            out_acc = [singles.tile([128, H], f32, tag=f"oacc{tt}",
                                    name=f"oacc{tt}") for tt in range(N_TT)]

            chunks = []
            for ei, e in enumerate(("f", "c", "s")):
                i_dim = w1[e].shape[1]
                for ic in range(i_dim // CHUNK_I):
                    chunks.append((ei, e, ic))

            for g, (ei, e, ic) in enumerate(chunks):
                w1r = w1[e].rearrange("(hi p) i -> p hi i", p=128)
                w2r = w2[e].rearrange("(blk p) h -> p blk h", p=128)

                w1t = []
                for hi in range(N_HI):
                    wt = w1p.tile([128, CHUNK_I], bf16, tag="w1t", name="w1t")
                    nc.sync.dma_start(
                        out=wt[:],
                        in_=w1r[:, hi, ic * CHUNK_I:(ic + 1) * CHUNK_I])
                    w1t.append(wt)
                    if g == 0:
                        # Interleave activation slices with the first chunk's
                        # weights: GEMM1 consumes (w1t[hi], xT[hi]) pairs.
                        nc.sync.dma_start(out=xT_sb[hi][:],
                                          in_=xT_r[:, hi, :])
                w2t = []
                for j in range(N_JB):
                    wt = w2p.tile([128, H], bf16, tag="w2t", name="w2t")
                    nc.sync.dma_start(
                        out=wt[:], in_=w2r[:, ic * N_JB + j, :])
                    w2t.append(wt)

                hT = hp.tile([128, N_JB, TOK], bf16, tag="hT")
                for j in range(N_JB):
                    blk = ic * N_JB + j
                    # Both token-halves accumulate in parallel PSUM banks so
                    # consecutive matmuls share the same stationary operand
                    # (one LDWEIGHTS per hi instead of per matmul).
                    ps1 = [ps1p.tile([128, 512], f32, tag="ps1", name="ps1")
                           for _ in range(N_TC)]
                    for hi in range(N_HI):
                        for t in range(N_TC):
                            nc.tensor.matmul(
                                ps1[t][:],
                                lhsT=w1t[hi][:, j * 128:(j + 1) * 128],
                                rhs=xT_sb[hi][:, t * 512:(t + 1) * 512],
                                start=(hi == 0),
                                stop=(hi == N_HI - 1),
                            )
                    for t in range(N_TC):
                        nc.scalar.activation(
                            out=hT[:, j, t * 512:(t + 1) * 512],
                            in_=ps1[t][:],
                            func=mybir.ActivationFunctionType.Silu,
                            bias=b1_sb[e][:, blk:blk + 1],
                            scale=1.0,
                        )

                for tt in range(N_TT):
                    # H-tiles processed in pairs: two PSUM banks accumulate
                    # while the stationary hT block is loaded once per
                    # (pair, j), and the pair's evictions overlap the next
                    # pair's matmuls.
                    for hp2 in range(N_HH // 2):
                        ps2 = [ps2p.tile([128, 512], f32, tag="ps2", name="ps2")
                               for _ in range(2)]
                        for j in range(N_JB):
                            for hx in range(2):
                                hh = hp2 * 2 + hx
                                nc.tensor.matmul(
                                    ps2[hx][:],
                                    lhsT=hT[:, j, tt * 128:(tt + 1) * 128],
                                    rhs=w2t[j][:, hh * 512:(hh + 1) * 512],
                                    start=(j == 0),
                                    stop=(j == N_JB - 1),
                                )
                        for hx in range(2):
                            hh = hp2 * 2 + hx
                            acc = out_acc[tt][:, hh * 512:(hh + 1) * 512]
                            other = (bias2_sb[:, hh * 512:(hh + 1) * 512]
                                     if g == 0 else acc)
                            nc.vector.scalar_tensor_tensor(
                                out=acc,
                                in0=ps2[hx][:],
                                scalar=wvec_sb[:, ei:ei + 1],
                                in1=other,
                                op0=mybir.AluOpType.mult,
                                op1=mybir.AluOpType.add,
                            )

            out_r = out.rearrange("(tt p) h -> p tt h", p=128)
            for tt in range(N_TT):
                nc.sync.dma_start(out=out_r[:, tt, :], in_=out_acc[tt][:])

    nc.compile()
    return nc


def _get_graph():
    if "nc" not in _compiled:
        _compiled["nc"] = _build_graph()
    return _compiled["nc"]


def _host_gate(hidden_states, gW1, gb1, gW2, gb2):
    """Exact mirror of the reference gate, in numpy f32. Returns w [B,3]."""
    gfeat = hidden_states.mean(axis=1, dtype=np.float32)        # [B,H]
    h1 = np.maximum(gfeat @ gW1 + gb1, 0.0).astype(np.float32)  # [B,H//4]
    glogit = (h1 @ gW2 + gb2).astype(np.float32)                # [B,3]
    z = glogit - glogit.max(axis=-1, keepdims=True)
    ez = np.exp(z)
    gw = (ez / ez.sum(axis=-1, keepdims=True)).astype(np.float32)
    sel = gw.argmax(axis=-1)
    maxw = gw.max(axis=-1)
    low_conf = maxw < CONF_THRESH
    onehot = np.eye(3, dtype=gw.dtype)[sel]
    return np.where(low_conf[:, None], gw, gw * onehot).astype(np.float32)


def _prepare_in_maps(inputs):
    w = _host_gate(inputs["hidden_states"], inputs["gW1"], inputs["gb1"],
                   inputs["gW2"], inputs["gb2"])

    shared = {
        "w1f": np.ascontiguousarray(inputs["fW1"].astype(BF16)),
        "w2f": np.ascontiguousarray(inputs["fW2"].astype(BF16)),
        "w1c": np.ascontiguousarray(inputs["cW1"].astype(BF16)),
        "w2c": np.ascontiguousarray(inputs["cW2"].astype(BF16)),
        "w1s": np.ascontiguousarray(inputs["sW1"].astype(BF16)),
        "w2s": np.ascontiguousarray(inputs["sW2"].astype(BF16)),
        "b1f": np.ascontiguousarray(
            inputs["fb1"].astype(np.float32).reshape(I_FULL // 128, 128).T),
        "b1c": np.ascontiguousarray(
            inputs["cb1"].astype(np.float32).reshape(I_COMP // 128, 128).T),
        "b1s": np.ascontiguousarray(
            inputs["sb1"].astype(np.float32).reshape(I_FULL // 128, 128).T),
    }

    x = inputs["hidden_states"]
    in_maps = []
    for core in range(N_CORES):
        b, half = core // 2, core % 2
        x_slice = x[b, half * TOK:(half + 1) * TOK, :]
        xT = np.ascontiguousarray(x_slice.T.astype(BF16))
        bias2 = (w[b, 0] * inputs["fb2"] + w[b, 1] * inputs["cb2"]
                 + w[b, 2] * inputs["sb2"]).astype(np.float32)
        wv = np.zeros((128, 4), np.float32)
        wv[:, :3] = w[b]
        in_maps.append({
            "xT": xT,
            "bias2": np.ascontiguousarray(
                np.broadcast_to(bias2, (128, H))),
            "wvec": wv,
            **shared,
        })
    return in_maps


def _run(inputs, trace=False):
    from concourse.bass_utils import run_bass_kernel_spmd

    nc = _get_graph()
    in_maps = _prepare_in_maps(inputs)
    res = run_bass_kernel_spmd(nc, in_maps, list(range(N_CORES)), trace=trace)

    x = inputs["hidden_states"]
    out = np.empty((B, S, H), np.float32)
    for core in range(N_CORES):
        b, half = core // 2, core % 2
        out[b, half * TOK:(half + 1) * TOK, :] = res.results[core]["out"]
    return out, res


def kernel(**inputs):
    out, _ = _run(inputs, trace=False)
    return out


# revision 14
# speedup vs baseline: 1.0087x; 1.0087x over previous
"""AdaptiveMLP (moe_routing) Trainium2 kernel — 8 NeuronCores, data-parallel.

Reference computation:
    gate = softmax(MLP_gate(mean_s(x)))          # [B,3] per-batch weights
    out  = sum_p w[b,p] * MLP_p(x[b])            # 3 expert MLPs (SiLU)
with w = gate masked to the argmax expert when confident (maxw >= 0.6).

Strategy:
  - The gate network is tiny ([4,2048] @ [2048,512] @ [512,3]); compute it on
    the host exactly as the reference does, producing the [B,3] combine
    weights `w`. The device then always computes all three expert MLPs and
    combines them with `w` — algebraically identical to the reference for
    both the confident and non-confident paths.
  - Data-parallel over tokens: 8 cores x 1024 tokens (core c takes batch
    c//2, sequence half c%2). No collectives; each core streams all expert
    weights once (bf16) while keeping its activations resident in SBUF.
  - bf16 compute on the TensorEngine (1 cycle/row vs 4 for fp32), f32
    accumulation in PSUM and for the output.

Per-core dataflow (tokens T=1024, H=2048, expert intermediate I in
{8192,4096,8192} processed in chunks of 512):
    xT [H,T] resident in SBUF (transposed+bf16 on host).
    per chunk: psum_h[i128,t512] = sum_hi W1[hi,i128].T @ xT[hi,t512]
               hT = Silu(psum_h + b1)                  (ScalarE, bf16)
               psum_o[t128,h512] = sum_j hT[j,t128].T @ W2[j,h512]
               out_acc += w[expert] * psum_o           (VectorE, f32)
    out_acc initialised with sum_p w[b,p]*b2_p (host-combined bias).
"""

import os
import sys

import numpy as np

sys.path.insert(0, "/opt/trn_rl_repo")

import ml_dtypes

BF16 = ml_dtypes.bfloat16

B, S, H = 4, 2048, 2048
I_FULL, I_COMP = 8192, 4096
N_CORES = 8
TOK = (B * S) // N_CORES  # 1024 tokens per core
CONF_THRESH = 0.6
CHUNK_I = 512  # intermediate-dim chunk processed per inner iteration

_compiled = {}


def _enable_ldw_opt():
    """Walrus can elide LDWEIGHTS for consecutive matmuls that share the
    same stationary operand, but concourse hardcodes --enable-ldw-opt=false.
    Rewrite the flag on the walrus command line; correctness is verified
    against the reference each run."""
    from concourse import bass_utils as _bu

    if getattr(_bu, "_ldw_opt_patched", False):
        return
    orig = _bu.run_command

    # NOTE: --enable-ldw-opt=true crashes this walrus build's codegen
    # (visitInstLdweights, CoreV3GenImpl.cpp:694) — leave the flag alone.
    _bu._ldw_opt_patched = True
    del orig


def _build_graph():
    import concourse.tile as tile
    from concourse import bacc, mybir

    _enable_ldw_opt()

    nc = bacc.Bacc("TRN2", target_bir_lowering=False, debug=False,
                   num_devices=N_CORES)
    f32 = mybir.dt.float32
    bf16 = mybir.dt.bfloat16

    xT = nc.dram_tensor("xT", [H, TOK], bf16, kind="ExternalInput").ap()
    w1 = {}
    w2 = {}
    for name, i_dim in (("f", I_FULL), ("c", I_COMP), ("s", I_FULL)):
        w1[name] = nc.dram_tensor(f"w1{name}", [H, i_dim], bf16,
                                  kind="ExternalInput").ap()
        w2[name] = nc.dram_tensor(f"w2{name}", [i_dim, H], bf16,
                                  kind="ExternalInput").ap()
    b1 = {
        "f": nc.dram_tensor("b1f", [128, I_FULL // 128], f32,
                            kind="ExternalInput").ap(),
        "c": nc.dram_tensor("b1c", [128, I_COMP // 128], f32,
                            kind="ExternalInput").ap(),
        "s": nc.dram_tensor("b1s", [128, I_FULL // 128], f32,
                            kind="ExternalInput").ap(),
    }
    bias2 = nc.dram_tensor("bias2", [128, H], f32, kind="ExternalInput").ap()
    wvec = nc.dram_tensor("wvec", [128, 4], f32, kind="ExternalInput").ap()
    out = nc.dram_tensor("out", [TOK, H], f32, kind="ExternalOutput").ap()

    N_HI = H // 128           # 16 contraction tiles for GEMM1
    N_HI_POOL = N_HI
    N_TT = TOK // 128         # 8 token tiles
    N_HH = H // 512           # 4 output-H tiles
    N_TC = TOK // 512         # 2 token chunks for GEMM1 moving operand
    N_JB = CHUNK_I // 128     # 4 I-blocks per chunk

    with tile.TileContext(nc) as tc:
        with (
            tc.tile_pool(name="singles", bufs=1) as singles,
            tc.tile_pool(name="w1p", bufs=2 * N_HI_POOL) as w1p,
            tc.tile_pool(name="w2p", bufs=2 * 4) as w2p,
            tc.tile_pool(name="hp", bufs=2) as hp,
            tc.tile_pool(name="ps1", bufs=4, space="PSUM") as ps1p,
            tc.tile_pool(name="ps2", bufs=4, space="PSUM") as ps2p,
        ):
            # Per-hi slice tiles so the first matmuls only wait on the
            # slices they read (head-latency: overlap DMA with compute).
            # The DMAs for xT are interleaved with the first chunk's W1
            # slices below so the first GEMM1 group starts ~3us in.
            xT_r = xT.rearrange("(hi p) t -> p hi t", p=128)
            xT_sb = [singles.tile([128, TOK], bf16, tag=f"xT{hi}",
                                  name=f"xT{hi}") for hi in range(N_HI)]
            bias2_sb = singles.tile([128, H], f32)
            nc.gpsimd.dma_start(out=bias2_sb[:], in_=bias2[:])
            wvec_sb = singles.tile([128, 4], f32)
            nc.gpsimd.dma_start(out=wvec_sb[:], in_=wvec[:])
            b1_sb = {}
            for e in ("f", "c", "s"):
                t = singles.tile([128, b1[e].shape[1]], f32, tag=f"b1{e}")
                nc.gpsimd.dma_start(out=t[:], in_=b1[e][:])
                b1_sb[e] = t
            out_acc = [singles.tile([128, H], f32, tag=f"oacc{tt}",
                                    name=f"oacc{tt}") for tt in range(N_TT)]

            chunks = []
            for ei, e in enumerate(("f", "c", "s")):
                i_dim = w1[e].shape[1]
                for ic in range(i_dim // CHUNK_I):
                    chunks.append((ei, e, ic))

            for g, (ei, e, ic) in enumerate(chunks):
                w1r = w1[e].rearrange("(hi p) i -> p hi i", p=128)
                w2r = w2[e].rearrange("(blk p) h -> p blk h", p=128)

                w1t = []
                for hi in range(N_HI):
                    wt = w1p.tile([128, CHUNK_I], bf16, tag="w1t", name="w1t")
                    nc.sync.dma_start(
                        out=wt[:],
                        in_=w1r[:, hi, ic * CHUNK_I:(ic + 1) * CHUNK_I])
                    w1t.append(wt)
                    if g == 0:
                        # Interleave activation slices with the first chunk's
                        # weights: GEMM1 consumes (w1t[hi], xT[hi]) pairs.
                        nc.sync.dma_start(out=xT_sb[hi][:],
                                          in_=xT_r[:, hi, :])
                w2t = []
                for j in range(N_JB):
                    wt = w2p.tile([128, H], bf16, tag="w2t", name="w2t")
                    nc.sync.dma_start(
                        out=wt[:], in_=w2r[:, ic * N_JB + j, :])
                    w2t.append(wt)

                hT = hp.tile([128, N_JB, TOK], bf16, tag="hT")
                for j in range(N_JB):
                    blk = ic * N_JB + j
                    # Both token-halves accumulate in parallel PSUM banks so
                    # consecutive matmuls share the same stationary operand
                    # (one LDWEIGHTS per hi instead of per matmul).
                    ps1 = [ps1p.tile([128, 512], f32, tag="ps1", name="ps1")
                           for _ in range(N_TC)]
                    for hi in range(N_HI):
                        for t in range(N_TC):
                            nc.tensor.matmul(
                                ps1[t][:],
                                lhsT=w1t[hi][:, j * 128:(j + 1) * 128],
                                rhs=xT_sb[hi][:, t * 512:(t + 1) * 512],
                                start=(hi == 0),
                                stop=(hi == N_HI - 1),
                            )
                    for t in range(N_TC):
                        nc.scalar.activation(
                            out=hT[:, j, t * 512:(t + 1) * 512],
                            in_=ps1[t][:],
                            func=mybir.ActivationFunctionType.Silu,
                            bias=b1_sb[e][:, blk:blk + 1],
                            scale=1.0,
                        )

                for tt in range(N_TT):
                    # H-tiles processed in pairs: two PSUM banks accumulate
                    # while the stationary hT block is loaded once per
                    # (pair, j), and the pair's evictions overlap the next
                    # pair's matmuls.
                    for hp2 in range(N_HH // 2):
                        ps2 = [ps2p.tile([128, 512], f32, tag="ps2", name="ps2")
                               for _ in range(2)]
                        for j in range(N_JB):
                            for hx in range(2):
                                hh = hp2 * 2 + hx
                                nc.tensor.matmul(
                                    ps2[hx][:],
                                    lhsT=hT[:, j, tt * 128:(tt + 1) * 128],
                                    rhs=w2t[j][:, hh * 512:(hh + 1) * 512],
                                    start=(j == 0),
                                    stop=(j == N_JB - 1),
                                )
                        for hx in range(2):
                            hh = hp2 * 2 + hx
                            acc = out_acc[tt][:, hh * 512:(hh + 1) * 512]
                            other = (bias2_sb[:, hh * 512:(hh + 1) * 512]
                                     if g == 0 else acc)
                            nc.vector.scalar_tensor_tensor(
                                out=acc,
                                in0=ps2[hx][:],
                                scalar=wvec_sb[:, ei:ei + 1],
                                in1=other,
                                op0=mybir.AluOpType.mult,
                                op1=mybir.AluOpType.add,
                            )

            out_r = out.rearrange("(tt p) h -> p tt h", p=128)
            for tt in range(N_TT):
                nc.sync.dma_start(out=out_r[:, tt, :], in_=out_acc[tt][:])

    nc.compile()
    return nc


def _get_graph():
    if "nc" not in _compiled:
        _compiled["nc"] = _build_graph()
    return _compiled["nc"]


def _host_gate(hidden_states, gW1, gb1, gW2, gb2):
    """Exact mirror of the reference gate, in numpy f32. Returns w [B,3]."""
    gfeat = hidden_states.mean(axis=1, dtype=np.float32)        # [B,H]
    h1 = np.maximum(gfeat @ gW1 + gb1, 0.0).astype(np.float32)  # [B,H//4]
    glogit = (h1 @ gW2 + gb2).astype(np.float32)                # [B,3]
    z = glogit - glogit.max(axis=-1, keepdims=True)
    ez = np.exp(z)
    gw = (ez / ez.sum(axis=-1, keepdims=True)).astype(np.float32)
    sel = gw.argmax(axis=-1)
    maxw = gw.max(axis=-1)
    low_conf = maxw < CONF_THRESH
    onehot = np.eye(3, dtype=gw.dtype)[sel]
    return np.where(low_conf[:, None], gw, gw * onehot).astype(np.float32)


def _prepare_in_maps(inputs):
    w = _host_gate(inputs["hidden_states"], inputs["gW1"], inputs["gb1"],
                   inputs["gW2"], inputs["gb2"])

    shared = {
        "w1f": np.ascontiguousarray(inputs["fW1"].astype(BF16)),
        "w2f": np.ascontiguousarray(inputs["fW2"].astype(BF16)),
        "w1c": np.ascontiguousarray(inputs["cW1"].astype(BF16)),
        "w2c": np.ascontiguousarray(inputs["cW2"].astype(BF16)),
        "w1s": np.ascontiguousarray(inputs["sW1"].astype(BF16)),
        "w2s": np.ascontiguousarray(inputs["sW2"].astype(BF16)),
        "b1f": np.ascontiguousarray(
            inputs["fb1"].astype(np.float32).reshape(I_FULL // 128, 128).T),
        "b1c": np.ascontiguousarray(
            inputs["cb1"].astype(np.float32).reshape(I_COMP // 128, 128).T),
        "b1s": np.ascontiguousarray(
            inputs["sb1"].astype(np.float32).reshape(I_FULL // 128, 128).T),
    }

    x = inputs["hidden_states"]
    in_maps = []
    for core in range(N_CORES):
        b, half = core // 2, core % 2
        x_slice = x[b, half * TOK:(half + 1) * TOK, :]
        xT = np.ascontiguousarray(x_slice.T.astype(BF16))
        bias2 = (w[b, 0] * inputs["fb2"] + w[b, 1] * inputs["cb2"]
                 + w[b, 2] * inputs["sb2"]).astype(np.float32)
        wv = np.zeros((128, 4), np.float32)
        wv[:, :3] = w[b]
        in_maps.append({
            "xT": xT,
            "bias2": np.ascontiguousarray(
                np.broadcast_to(bias2, (128, H))),
            "wvec": wv,
            **shared,
        })
    return in_maps


def _run(inputs, trace=False):
    from concourse.bass_utils import run_bass_kernel_spmd

    nc = _get_graph()
    in_maps = _prepare_in_maps(inputs)
    res = run_bass_kernel_spmd(nc, in_maps, list(range(N_CORES)), trace=trace)

    x = inputs["hidden_states"]
    out = np.empty((B, S, H), np.float32)
    for core in range(N_CORES):
        b, half = core // 2, core % 2
        out[b, half * TOK:(half + 1) * TOK, :] = res.results[core]["out"]
    return out, res


def kernel(**inputs):
    out, _ = _run(inputs, trace=False)
    return out


# revision 15
# speedup vs baseline: 1.0095x; 1.0009x over previous
"""AdaptiveMLP (moe_routing) Trainium2 kernel — 8 NeuronCores, data-parallel.

Reference computation:
    gate = softmax(MLP_gate(mean_s(x)))          # [B,3] per-batch weights
    out  = sum_p w[b,p] * MLP_p(x[b])            # 3 expert MLPs (SiLU)
with w = gate masked to the argmax expert when confident (maxw >= 0.6).

Strategy:
  - The gate network is tiny ([4,2048] @ [2048,512] @ [512,3]); compute it on
    the host exactly as the reference does, producing the [B,3] combine
    weights `w`. The device then always computes all three expert MLPs and
    combines them with `w` — algebraically identical to the reference for
    both the confident and non-confident paths.
  - Data-parallel over tokens: 8 cores x 1024 tokens (core c takes batch
    c//2, sequence half c%2). No collectives; each core streams all expert
    weights once (bf16) while keeping its activations resident in SBUF.
  - bf16 compute on the TensorEngine (1 cycle/row vs 4 for fp32), f32
    accumulation in PSUM and for the output.

Per-core dataflow (tokens T=1024, H=2048, expert intermediate I in
{8192,4096,8192} processed in chunks of 512):
    xT [H,T] resident in SBUF (transposed+bf16 on host).
    per chunk: psum_h[i128,t512] = sum_hi W1[hi,i128].T @ xT[hi,t512]
               hT = Silu(psum_h + b1)                  (ScalarE, bf16)
               psum_o[t128,h512] = sum_j hT[j,t128].T @ W2[j,h512]
               out_acc += w[expert] * psum_o           (VectorE, f32)
    out_acc initialised with sum_p w[b,p]*b2_p (host-combined bias).
"""

import os
import sys

import numpy as np

sys.path.insert(0, "/opt/trn_rl_repo")

import ml_dtypes

BF16 = ml_dtypes.bfloat16

B, S, H = 4, 2048, 2048
I_FULL, I_COMP = 8192, 4096
N_CORES = 8
TOK = (B * S) // N_CORES  # 1024 tokens per core
CONF_THRESH = 0.6
CHUNK_I = 512  # intermediate-dim chunk processed per inner iteration

_compiled = {}


def _enable_ldw_opt():
    """Walrus can elide LDWEIGHTS for consecutive matmuls that share the
    same stationary operand, but concourse hardcodes --enable-ldw-opt=false.
    Rewrite the flag on the walrus command line; correctness is verified
    against the reference each run."""
    from concourse import bass_utils as _bu

    if getattr(_bu, "_ldw_opt_patched", False):
        return
    orig = _bu.run_command

    # NOTE: --enable-ldw-opt=true crashes this walrus build's codegen
    # (visitInstLdweights, CoreV3GenImpl.cpp:694) — leave the flag alone.
    _bu._ldw_opt_patched = True
    del orig


def _build_graph():
    import concourse.tile as tile
    from concourse import bacc, mybir

    _enable_ldw_opt()

    nc = bacc.Bacc("TRN2", target_bir_lowering=False, debug=False,
                   num_devices=N_CORES)
    f32 = mybir.dt.float32
    bf16 = mybir.dt.bfloat16

    xT = nc.dram_tensor("xT", [H, TOK], bf16, kind="ExternalInput").ap()
    w1 = {}
    w2 = {}
    for name, i_dim in (("f", I_FULL), ("c", I_COMP), ("s", I_FULL)):
        w1[name] = nc.dram_tensor(f"w1{name}", [H, i_dim], bf16,
                                  kind="ExternalInput").ap()
        w2[name] = nc.dram_tensor(f"w2{name}", [i_dim, H], bf16,
                                  kind="ExternalInput").ap()
    b1 = {
        "f": nc.dram_tensor("b1f", [128, I_FULL // 128], f32,
                            kind="ExternalInput").ap(),
        "c": nc.dram_tensor("b1c", [128, I_COMP // 128], f32,
                            kind="ExternalInput").ap(),
        "s": nc.dram_tensor("b1s", [128, I_FULL // 128], f32,
                            kind="ExternalInput").ap(),
    }
    bias2 = nc.dram_tensor("bias2", [128, H], f32, kind="ExternalInput").ap()
    wvec = nc.dram_tensor("wvec", [128, 4], f32, kind="ExternalInput").ap()
    out = nc.dram_tensor("out", [TOK, H], f32, kind="ExternalOutput").ap()

    N_HI = H // 128           # 16 contraction tiles for GEMM1
    N_HI_POOL = N_HI
    N_TT = TOK // 128         # 8 token tiles
    N_HH = H // 512           # 4 output-H tiles
    N_TC = TOK // 512         # 2 token chunks for GEMM1 moving operand
    N_JB = CHUNK_I // 128     # 4 I-blocks per chunk

    with tile.TileContext(nc) as tc:
        with (
            tc.tile_pool(name="singles", bufs=1) as singles,
            tc.tile_pool(name="w1p", bufs=2 * N_HI_POOL) as w1p,
            tc.tile_pool(name="w2p", bufs=2 * 4) as w2p,
            tc.tile_pool(name="hp", bufs=2) as hp,
            tc.tile_pool(name="ps1", bufs=4, space="PSUM") as ps1p,
            tc.tile_pool(name="ps2", bufs=4, space="PSUM") as ps2p,
        ):
            # Per-hi slice tiles so the first matmuls only wait on the
            # slices they read (head-latency: overlap DMA with compute).
            # The DMAs for xT are interleaved with the first chunk's W1
            # slices below so the first GEMM1 group starts ~3us in.
            xT_r = xT.rearrange("(hi p) t -> p hi t", p=128)
            xT_sb = [singles.tile([128, TOK], bf16, tag=f"xT{hi}",
                                  name=f"xT{hi}") for hi in range(N_HI)]
            bias2_sb = singles.tile([128, H], f32)
            nc.gpsimd.dma_start(out=bias2_sb[:], in_=bias2[:])
            wvec_sb = singles.tile([128, 4], f32)
            nc.gpsimd.dma_start(out=wvec_sb[:], in_=wvec[:])
            b1_sb = {}
            for e in ("f", "c", "s"):
                t = singles.tile([128, b1[e].shape[1]], f32, tag=f"b1{e}")
                nc.gpsimd.dma_start(out=t[:], in_=b1[e][:])
                b1_sb[e] = t
            out_acc = [singles.tile([128, H], f32, tag=f"oacc{tt}",
                                    name=f"oacc{tt}") for tt in range(N_TT)]

            chunks = []
            for ei, e in enumerate(("f", "c", "s")):
                i_dim = w1[e].shape[1]
                for ic in range(i_dim // CHUNK_I):
                    chunks.append((ei, e, ic))

            for g, (ei, e, ic) in enumerate(chunks):
                w1r = w1[e].rearrange("(hi p) i -> p hi i", p=128)
                w2r = w2[e].rearrange("(blk p) h -> p blk h", p=128)

                w1t = []
                for hi in range(N_HI):
                    wt = w1p.tile([128, CHUNK_I], bf16, tag="w1t", name="w1t")
                    nc.sync.dma_start(
                        out=wt[:],
                        in_=w1r[:, hi, ic * CHUNK_I:(ic + 1) * CHUNK_I])
                    w1t.append(wt)
                    if g == 0:
                        # Interleave activation slices with the first chunk's
                        # weights: GEMM1 consumes (w1t[hi], xT[hi]) pairs.
                        nc.sync.dma_start(out=xT_sb[hi][:],
                                          in_=xT_r[:, hi, :])
                w2t = []
                for j in range(N_JB):
                    wt = w2p.tile([128, H], bf16, tag="w2t", name="w2t")
                    nc.sync.dma_start(
                        out=wt[:], in_=w2r[:, ic * N_JB + j, :])
                    w2t.append(wt)

                hT = hp.tile([128, N_JB, TOK], bf16, tag="hT")
                for j in range(N_JB):
                    blk = ic * N_JB + j
                    # Both token-halves accumulate in parallel PSUM banks so
                    # consecutive matmuls share the same stationary operand
                    # (one LDWEIGHTS per hi instead of per matmul).
                    ps1 = [ps1p.tile([128, 512], f32, tag="ps1", name="ps1")
                           for _ in range(N_TC)]
                    for hi in range(N_HI):
                        for t in range(N_TC):
                            nc.tensor.matmul(
                                ps1[t][:],
                                lhsT=w1t[hi][:, j * 128:(j + 1) * 128],
                                rhs=xT_sb[hi][:, t * 512:(t + 1) * 512],
                                start=(hi == 0),
                                stop=(hi == N_HI - 1),
                            )
                    for t in range(N_TC):
                        nc.scalar.activation(
                            out=hT[:, j, t * 512:(t + 1) * 512],
                            in_=ps1[t][:],
                            func=mybir.ActivationFunctionType.Silu,
                            bias=b1_sb[e][:, blk:blk + 1],
                            scale=1.0,
                        )

                for tt in range(N_TT):
                    # H-tiles processed in pairs: two PSUM banks accumulate
                    # while the stationary hT block is loaded once per
                    # (pair, j), and the pair's evictions overlap the next
                    # pair's matmuls.
                    for hp2 in range(N_HH // 2):
                        ps2 = [ps2p.tile([128, 512], f32, tag="ps2", name="ps2")
                               for _ in range(2)]
                        for j in range(N_JB):
                            for hx in range(2):
                                hh = hp2 * 2 + hx
                                nc.tensor.matmul(
                                    ps2[hx][:],
                                    lhsT=hT[:, j, tt * 128:(tt + 1) * 128],
                                    rhs=w2t[j][:, hh * 512:(hh + 1) * 512],
                                    start=(j == 0),
                                    stop=(j == N_JB - 1),
                                )
                        for hx in range(2):
                            hh = hp2 * 2 + hx
                            acc = out_acc[tt][:, hh * 512:(hh + 1) * 512]
                            other = (bias2_sb[:, hh * 512:(hh + 1) * 512]
                                     if g == 0 else acc)
                            nc.vector.scalar_tensor_tensor(
                                out=acc,
                                in0=ps2[hx][:],
                                scalar=wvec_sb[:, ei:ei + 1],
                                in1=other,
                                op0=mybir.AluOpType.mult,
                                op1=mybir.AluOpType.add,
                            )

            out_r = out.rearrange("(tt p) h -> p tt h", p=128)
            for tt in range(N_TT):
                nc.sync.dma_start(out=out_r[:, tt, :], in_=out_acc[tt][:])

    nc.compile()
    return nc


def _get_graph():
    if "nc" not in _compiled:
        _compiled["nc"] = _build_graph()
    return _compiled["nc"]


def _host_gate(hidden_states, gW1, gb1, gW2, gb2):
    """Exact mirror of the reference gate, in numpy f32. Returns w [B,3]."""
    gfeat = hidden_states.mean(axis=1, dtype=np.float32)        # [B,H]
    h1 = np.maximum(gfeat @ gW1 + gb1, 0.0).astype(np.float32)  # [B,H//4]
    glogit = (h1 @ gW2 + gb2).astype(np.float32)                # [B,3]
    z = glogit - glogit.max(axis=-1, keepdims=True)
    ez = np.exp(z)
    gw = (ez / ez.sum(axis=-1, keepdims=True)).astype(np.float32)
    sel = gw.argmax(axis=-1)
    maxw = gw.max(axis=-1)
    low_conf = maxw < CONF_THRESH
    onehot = np.eye(3, dtype=gw.dtype)[sel]
    return np.where(low_conf[:, None], gw, gw * onehot).astype(np.float32)


def _prepare_in_maps(inputs):
    w = _host_gate(inputs["hidden_states"], inputs["gW1"], inputs["gb1"],
                   inputs["gW2"], inputs["gb2"])

    shared = {
        "w1f": np.ascontiguousarray(inputs["fW1"].astype(BF16)),
        "w2f": np.ascontiguousarray(inputs["fW2"].astype(BF16)),
        "w1c": np.ascontiguousarray(inputs["cW1"].astype(BF16)),
        "w2c": np.ascontiguousarray(inputs["cW2"].astype(BF16)),
        "w1s": np.ascontiguousarray(inputs["sW1"].astype(BF16)),
        "w2s": np.ascontiguousarray(inputs["sW2"].astype(BF16)),
        "b1f": np.ascontiguousarray(
            inputs["fb1"].astype(np.float32).reshape(I_FULL // 128, 128).T),
        "b1c": np.ascontiguousarray(
            inputs["cb1"].astype(np.float32).reshape(I_COMP // 128, 128).T),
        "b1s": np.ascontiguousarray(
            inputs["sb1"].astype(np.float32).reshape(I_FULL // 128, 128).T),
    }

    x = inputs["hidden_states"]
    in_maps = []
    for core in range(N_CORES):
        b, half = core // 2, core % 2
        x_slice = x[b, half * TOK:(half + 1) * TOK, :]
        xT = np.ascontiguousarray(x_slice.T.astype(BF16))
        bias2 = (w[b, 0] * inputs["fb2"] + w[b, 1] * inputs["cb2"]
                 + w[b, 2] * inputs["sb2"]).astype(np.float32)
        wv = np.zeros((128, 4), np.float32)
        wv[:, :3] = w[b]
        in_maps.append({
            "xT": xT,
            "bias2": np.ascontiguousarray(
                np.broadcast_to(bias2, (128, H))),
            "wvec": wv,
            **shared,
        })
    return in_maps


def _spot_check(inputs, out, n_tokens=2):
    """Cheap host-side bf16-sim check of a few tokens per batch. Catches
    transient device corruption (observed rarely as garbage output rows)."""
    rng = np.random.default_rng(12345)
    w = _host_gate(inputs["hidden_states"], inputs["gW1"], inputs["gb1"],
                   inputs["gW2"], inputs["gb2"])
    f32 = np.float32
    for b in range(B):
        s_idx = rng.integers(0, S, size=n_tokens)
        xs = inputs["hidden_states"][b, s_idx, :].astype(BF16).astype(f32)
        ref = np.zeros((n_tokens, H), f32)
        for ei, (W1n, b1n, W2n, b2n) in enumerate(
                (("fW1", "fb1", "fW2", "fb2"),
                 ("cW1", "cb1", "cW2", "cb2"),
                 ("sW1", "sb1", "sW2", "sb2"))):
            W1 = inputs[W1n].astype(BF16).astype(f32)
            W2 = inputs[W2n].astype(BF16).astype(f32)
            z = xs @ W1 + inputs[b1n]
            h = (z / (1.0 + np.exp(-z))).astype(BF16).astype(f32)
            ref += w[b, ei] * (h @ W2 + inputs[b2n])
        got = out[b, s_idx, :]
        rel = (np.linalg.norm(got - ref)
               / max(np.linalg.norm(ref), 1e-20))
        if not np.isfinite(rel) or rel > 5e-3:
            return False
    return True


def _run(inputs, trace=False):
    from concourse.bass_utils import run_bass_kernel_spmd

    nc = _get_graph()
    in_maps = _prepare_in_maps(inputs)

    last_err = None
    for attempt in range(3):
        try:
            res = run_bass_kernel_spmd(nc, in_maps, list(range(N_CORES)),
                                       trace=trace)
        except Exception as exc:  # transient device errors — retry
            last_err = exc
            import time
            time.sleep(10 * (attempt + 1))
            continue
        out = np.empty((B, S, H), np.float32)
        for core in range(N_CORES):
            b, half = core // 2, core % 2
            out[b, half * TOK:(half + 1) * TOK, :] = res.results[core]["out"]
        if _spot_check(inputs, out):
            return out, res
        last_err = RuntimeError("spot check failed (corrupt device output)")
    raise last_err


def kernel(**inputs):
    out, _ = _run(inputs, trace=False)
    return out
